# revision 44
# baseline (speedup 1.0000x reference)
"""Bidirectional toroidal lattice message passing on 8 Trainium2 cores.

The [N,N] adjacencies produced by this model are toroidal 3-neighbor shift
operators (3 constant-value generalized diagonals, zero elsewhere). We verify
that structure on the host, extract the per-shift constants, and run the
10-step propagation fully on-chip:

  - partition dim = theta (128); free dims = (dir 2, batch 2, phi 64+halo)
  - theta-shift terms via two 128x128 stationary matmuls per direction
    accumulated in PSUM:  P_d = v10*Ptheta_d  and  M_d = v11*Ptheta_d + v01*I
    applied to x and to the phi-shifted view of x
  - phi-shift is free: x tiles carry 2 halo columns kept up to date with two
    tiny copies per step, so the shifted operand is just a strided view
  - per-step DVE: tmp = psum * g_e ; x' = c1*x + tmp   (g_e = 0.7*decay*af
    host-prepacked); step accumulation acc += w_s * x' runs on GPSIMD

Batch is sharded 2-per-core across 8 cores; no collectives needed.
"""

import numpy as np

NT, NP, S = 128, 64, 10
N = NT * NP
B = 16
NCORES = 8
BPC = B // NCORES  # batches per core
NH = NP + 2        # phi width incl. wrap halos: [wrap_pre | 0..63 | wrap_post]

_FWD = [(1, 0), (0, 1), (1, 1)]
_REV = [(-1, 0), (0, -1), (-1, -1)]


def _diag_vals(adj, shifts):
    idx = np.arange(N)
    ti, pi = idx // NP, idx % NP
    return [adj[idx, ((ti + dt) % NT) * NP + (pi + dp) % NP] for dt, dp in shifts]


def _softmax(x):
    e = np.exp(x - x.max())
    return (e / e.sum()).astype(np.float32)


def _structure_ok(adj, vals):
    for v in vals:
        if np.ptp(v) > 1e-6 * max(1.0, abs(float(v.mean()))):
            return False
    total = adj.sum(dtype=np.float64)
    diag = sum(v.sum(dtype=np.float64) for v in vals)
    return abs(total - diag) < 1e-3


def _reference_fallback(entry, fwd_adj, rev_adj, fwd_sw, fwd_decay, rev_sw,
                        rev_decay, iw, angles):
    # generic dense path (host); only used if the adjacency is not the
    # expected toroidal shift structure.
    def prop(adj, decay, sw):
        d = float(np.clip(decay, 0.5, 0.99))
        af = 0.5 + 0.5 * np.cos(np.abs(angles).mean(axis=1))
        x = entry.astype(np.float32)
        w = _softmax(np.asarray(sw, np.float32))
        acc = np.zeros_like(x)
        for s in range(S):
            p = (x @ adj) * af[None, :]
            x = ((0.3 * x + 0.7 * p) * d).astype(np.float32)
            acc += w[s] * x
        return acc
    f = prop(fwd_adj, fwd_decay, fwd_sw)
    r = prop(rev_adj, rev_decay, rev_sw)
    inter = f * r
    sig = 1.0 / (1.0 + np.exp(-float(iw)))
    return (f + r + np.float32(sig) * inter).astype(np.float32), inter.astype(np.float32)


def _build_program(c1, w, sig_w, use_fp32r=False):
    """SPMD Bass program (identical on all cores).

    consts layout (free dim, fp32):  [g_e 2*BPC*64 | x0 BPC*NH]
    mats_d (uint8): the four 0/1 shift matrices [P_f|M_f|P_r|M_r], cast to
    fp32 on device; the per-shift constant v is folded into g_e on the host.
    """
    import concourse.bacc as bacc
    import concourse.mybir as mybir
    from concourse.tile import TileContext

    fp32 = mybir.dt.float32
    u8 = mybir.dt.uint8
    mm_dt = mybir.dt.float32r if use_fp32r else fp32
    OP = mybir.AluOpType

    nc = bacc.Bacc(None, target_bir_lowering=False)

    GE = 2 * BPC * NP
    XW = BPC * NH          # x0 stored once; both directions read the same view
    CW = GE + XW
    consts_d = nc.dram_tensor("consts", [NT, CW], fp32, kind="ExternalInput")
    mats_d = nc.dram_tensor("mats", [NT, 4 * NT], u8, kind="ExternalInput")
    out_d = nc.dram_tensor("out_all", [2, BPC, N], fp32, kind="ExternalOutput")

    (c1f, c1r), (wf, wr) = c1, w
    same_c1 = abs(c1f - c1r) < 1e-12
    same_w = all(abs(a - b) < 1e-12 for a, b in zip(wf, wr))

    g_off = 0
    x0_off = g_off + GE

    with TileContext(nc) as tc:
        with (
            tc.tile_pool(name="const", bufs=1) as cpool,
            tc.tile_pool(name="state", bufs=3) as spool,
            tc.tile_pool(name="work", bufs=3) as wpool,
            tc.tile_pool(name="accp", bufs=2) as apool,
            tc.tile_pool(name="psum", bufs=4, space="PSUM") as ppool,
        ):
            # constant loads issued from five different engine queues so the
            # issues don't serialize and the transfers run in parallel
            consts = cpool.tile([NT, CW], fp32, tag="consts")
            mats8 = cpool.tile([NT, 4 * NT], u8, tag="mats8")
            matsf = cpool.tile([NT, 4 * NT], fp32, tag="matsf")
            # per-matrix DMAs on alternating queues + per-matrix casts, in
            # first-use order: Pf, Pr, Mf, Mr
            nc.scalar.dma_start(consts[:, x0_off:x0_off + XW],
                                consts_d[:, x0_off:x0_off + XW])
            for i, q in ((0, nc.sync), (2, nc.scalar), (1, nc.sync), (3, nc.scalar)):
                q.dma_start(mats8[:, i * NT:(i + 1) * NT],
                            mats_d[:, i * NT:(i + 1) * NT])
                nc.vector.tensor_copy(matsf[:, i * NT:(i + 1) * NT],
                                      mats8[:, i * NT:(i + 1) * NT])
            nc.sync.dma_start(consts[:, g_off:g_off + GE],
                              consts_d[:, g_off:g_off + GE])
            pm = {k: matsf[:, i * NT:(i + 1) * NT].bitcast(mm_dt)
                  for i, k in enumerate(("Pf", "Mf", "Pr", "Mr"))}
            g_e = consts[:, g_off:g_off + GE].rearrange(
                "t (d b p) -> t d b p", d=2, b=BPC)

            # initial state: host-packed with halos, read in place
            x0 = consts[:, x0_off:x0_off + XW].rearrange(
                "t (b p) -> t b p", b=BPC)

            # fully independent fwd / rev chains so one direction's DVE work
            # overlaps the other direction's matmuls
            DIRS = (
                dict(d=0, P="Pf", M="Mf", c1=c1f, w=wf, xtag="xf"),
                dict(d=1, P="Pr", M="Mr", c1=c1r, w=wr, xtag="xr"),
            )
            xs = [x0, x0]                     # [128, BPC, NH] views
            accs = [None, None]
            for s in range(S):
                for dd in DIRS:
                    d = dd["d"]
                    xd = xs[d]
                    # prop = P @ x + M @ xphi  (xphi: fwd cols 0..63, rev 2..65)
                    ps = ppool.tile([NT, BPC, NP], fp32, tag=f"ps{d}")
                    xm = xd.bitcast(mm_dt)
                    lo = 0 if d == 0 else 2
                    nc.tensor.matmul(ps[:], pm[dd["P"]], xm[:, :, 1:NP + 1],
                                     start=True, stop=False)
                    nc.tensor.matmul(ps[:], pm[dd["M"]], xm[:, :, lo:lo + NP],
                                     start=False, stop=True)

                    # tmp = prop * g_e ; x'_center = c1*x + tmp
                    tmp = wpool.tile([NT, BPC, NP], fp32, tag=f"tmp{d}")
                    nc.vector.tensor_mul(tmp[:], ps[:], g_e[:, d])
                    xn = spool.tile([NT, BPC, NH], fp32, tag=dd["xtag"])
                    xc = xn[:, :, 1:NP + 1]
                    nc.vector.scalar_tensor_tensor(
                        xc, xd[:, :, 1:NP + 1], dd["c1"], tmp[:],
                        op0=OP.mult, op1=OP.add)
                    # refresh wrap halos (tiny; last step never reads them)
                    if s < S - 1:
                        nc.gpsimd.tensor_copy(xn[:, :, 0:1], xn[:, :, NP:NP + 1])
                        nc.gpsimd.tensor_copy(xn[:, :, NP + 1:NP + 2], xn[:, :, 1:2])
                    xs[d] = xn[:]

                    # acc += w_s * x'
                    an = apool.tile([NT, BPC, NP], fp32, tag=f"acc{d}")
                    if accs[d] is None:
                        nc.vector.tensor_scalar_mul(an[:], xc, dd["w"][0])
                    else:
                        nc.vector.scalar_tensor_tensor(
                            an[:], xc, dd["w"][s], accs[d][:],
                            op0=OP.mult, op1=OP.add)
                    accs[d] = an

            # combine: inter = f*r ; comb = f + r + sig*inter
            # two output DMAs on different queues, each fired as soon as its
            # tensor is ready
            out2 = wpool.tile([NT, 2, BPC, NP], fp32, tag="out2")
            inter = out2[:, 1]
            nc.vector.tensor_mul(inter, accs[0][:], accs[1][:])
            fr = wpool.tile([NT, BPC, NP], fp32, tag="fr")
            nc.vector.tensor_add(fr[:], accs[0][:], accs[1][:])
            ov = out_d[:].rearrange("o b (t p) -> o t b p", t=NT)
            nc.scalar.dma_start(ov[1], inter)
            nc.vector.scalar_tensor_tensor(
                out2[:, 0], inter, sig_w, fr[:], op0=OP.mult, op1=OP.add)
            nc.sync.dma_start(ov[0], out2[:, 0])

    nc.finalize()
    return nc


def _host_prep(inputs):
    entry = np.ascontiguousarray(np.asarray(inputs["entry_probs"], np.float32))
    fwd_adj = np.asarray(inputs["forward_adj"], np.float32)
    rev_adj = np.asarray(inputs["reverse_adj"], np.float32)
    angles = np.asarray(inputs["bounce_angles"], np.float32)

    vf = _diag_vals(fwd_adj, _FWD)
    vr = _diag_vals(rev_adj, _REV)
    ok = _structure_ok(fwd_adj, vf) and _structure_ok(rev_adj, vr)

    df = float(np.clip(float(np.asarray(inputs["forward_decay"])), 0.5, 0.99))
    dr = float(np.clip(float(np.asarray(inputs["reverse_decay"])), 0.5, 0.99))
    wf = _softmax(np.asarray(inputs["forward_step_weights"], np.float32))
    wr = _softmax(np.asarray(inputs["reverse_step_weights"], np.float32))
    sig = float(1.0 / (1.0 + np.exp(-float(np.asarray(inputs["interaction_weight"])))))

    vbf = [float(v.mean()) for v in vf]   # [v10, v01, v11]
    vbr = [float(v.mean()) for v in vr]
    # 0/1 uint8 matrices require one shared constant per direction
    for vs in (vbf, vbr):
        if abs(vs[0] - vs[1]) > 1e-6 * abs(vs[0]) or \
           abs(vs[0] - vs[2]) > 1e-6 * abs(vs[0]):
            ok = False

    k = np.arange(NT)
    p_fwd = np.zeros((NT, NT), np.uint8)   # out[t] = in[t-1]
    p_fwd[(k - 1) % NT, k] = 1
    p_rev = np.zeros((NT, NT), np.uint8)   # out[t] = in[t+1]
    p_rev[(k + 1) % NT, k] = 1
    eye = np.eye(NT, dtype=np.uint8)

    mats_u8 = np.ascontiguousarray(np.concatenate(
        [p_fwd, p_fwd + eye, p_rev, p_rev + eye], axis=1))

    # per-cell gain g = v * 0.7 * decay * (0.5 + 0.5*cos(mean|angles|)),
    # expanded to [128, dir, b, 64]; v folded here since matrices are 0/1
    af = (0.5 + 0.5 * np.cos(np.abs(angles).mean(axis=1))).astype(np.float32)
    af2 = af.reshape(NT, NP)
    g_e = np.empty((NT, 2, BPC, NP), np.float32)
    g_e[:, 0] = (0.7 * df * vbf[0]) * af2[:, None, :]
    g_e[:, 1] = (0.7 * dr * vbr[0]) * af2[:, None, :]

    consts_common = g_e.reshape(NT, -1)

    consts_list = []
    for c in range(NCORES):
        e = entry[c * BPC:(c + 1) * BPC].reshape(BPC, NT, NP).transpose(1, 0, 2)
        x0 = np.empty((NT, BPC, NH), np.float32)
        x0[:, :, 1:NP + 1] = e
        x0[:, :, 0] = e[:, :, NP - 1]
        x0[:, :, NP + 1] = e[:, :, 0]
        consts_list.append(np.ascontiguousarray(
            np.concatenate([consts_common, x0.reshape(NT, -1)], axis=1)))

    meta = dict(
        ok=ok,
        c1=(0.3 * df, 0.3 * dr),
        w=(list(map(float, wf)), list(map(float, wr))),
        sig=sig, consts_list=consts_list, mats_u8=mats_u8,
    )
    return meta


_PROGRAM_CACHE = {}
LAST_RESULT = None
USE_FP32R = False


def kernel(**inputs):
    meta = _host_prep(inputs)
    if not meta["ok"]:
        return _reference_fallback(
            np.asarray(inputs["entry_probs"], np.float32),
            np.asarray(inputs["forward_adj"], np.float32),
            np.asarray(inputs["reverse_adj"], np.float32),
            inputs["forward_step_weights"], inputs["forward_decay"],
            inputs["reverse_step_weights"], inputs["reverse_decay"],
            inputs["interaction_weight"], np.asarray(inputs["bounce_angles"], np.float32))

    from concourse import bass_utils

    key = (tuple(meta["c1"]), tuple(meta["w"][0]), tuple(meta["w"][1]),
           meta["sig"], USE_FP32R)
    if key not in _PROGRAM_CACHE:
        _PROGRAM_CACHE[key] = _build_program(
            meta["c1"], meta["w"], meta["sig"], use_fp32r=USE_FP32R)
    nc = _PROGRAM_CACHE[key]

    in_maps = [{"consts": meta["consts_list"][c], "mats": meta["mats_u8"]}
               for c in range(NCORES)]
    res = bass_utils.run_bass_kernel_spmd(nc, in_maps, core_ids=list(range(NCORES)))
    global LAST_RESULT
    LAST_RESULT = res

    combined = np.concatenate([r["out_all"][0] for r in res.results], axis=0)
    interaction = np.concatenate([r["out_all"][1] for r in res.results], axis=0)
    return combined, interaction


# revision 50
# speedup vs baseline: 1.0411x; 1.0411x over previous
"""Bidirectional toroidal lattice message passing on 8 Trainium2 cores.

The [N,N] adjacencies produced by this model are toroidal 3-neighbor shift
operators (3 constant-value generalized diagonals, zero elsewhere). We verify
that structure on the host, extract the per-shift constants, and run the
10-step propagation fully on-chip:

  - partition dim = theta (128); free dims = (dir 2, batch 2, phi 64+halo)
  - theta-shift terms via two 128x128 stationary matmuls per direction
    accumulated in PSUM:  P_d = v10*Ptheta_d  and  M_d = v11*Ptheta_d + v01*I
    applied to x and to the phi-shifted view of x
  - phi-shift is free: x tiles carry 2 halo columns kept up to date with two
    tiny copies per step, so the shifted operand is just a strided view
  - per-step DVE: tmp = psum * g_e ; x' = c1*x + tmp   (g_e = 0.7*decay*af
    host-prepacked); step accumulation acc += w_s * x' runs on GPSIMD

Batch is sharded 2-per-core across 8 cores; no collectives needed.
"""

import numpy as np

NT, NP, S = 128, 64, 10
N = NT * NP
B = 16
NCORES = 8
BPC = B // NCORES  # batches per core
NH = NP + 2        # phi width incl. wrap halos: [wrap_pre | 0..63 | wrap_post]

_FWD = [(1, 0), (0, 1), (1, 1)]
_REV = [(-1, 0), (0, -1), (-1, -1)]


def _diag_vals(adj, shifts):
    idx = np.arange(N)
    ti, pi = idx // NP, idx % NP
    return [adj[idx, ((ti + dt) % NT) * NP + (pi + dp) % NP] for dt, dp in shifts]


def _softmax(x):
    e = np.exp(x - x.max())
    return (e / e.sum()).astype(np.float32)


def _structure_ok(adj, vals):
    for v in vals:
        if np.ptp(v) > 1e-6 * max(1.0, abs(float(v.mean()))):
            return False
    total = adj.sum(dtype=np.float64)
    diag = sum(v.sum(dtype=np.float64) for v in vals)
    return abs(total - diag) < 1e-3


def _reference_fallback(entry, fwd_adj, rev_adj, fwd_sw, fwd_decay, rev_sw,
                        rev_decay, iw, angles):
    # generic dense path (host); only used if the adjacency is not the
    # expected toroidal shift structure.
    def prop(adj, decay, sw):
        d = float(np.clip(decay, 0.5, 0.99))
        af = 0.5 + 0.5 * np.cos(np.abs(angles).mean(axis=1))
        x = entry.astype(np.float32)
        w = _softmax(np.asarray(sw, np.float32))
        acc = np.zeros_like(x)
        for s in range(S):
            p = (x @ adj) * af[None, :]
            x = ((0.3 * x + 0.7 * p) * d).astype(np.float32)
            acc += w[s] * x
        return acc
    f = prop(fwd_adj, fwd_decay, fwd_sw)
    r = prop(rev_adj, rev_decay, rev_sw)
    inter = f * r
    sig = 1.0 / (1.0 + np.exp(-float(iw)))
    return (f + r + np.float32(sig) * inter).astype(np.float32), inter.astype(np.float32)


def _build_program(c1, w, sig_w, use_fp32r=False):
    """SPMD Bass program (identical on all cores).

    consts layout (free dim, fp32):  [g_e 2*BPC*64 | x0 BPC*NH]
    mats_d (uint8): the four 0/1 shift matrices [P_f|M_f|P_r|M_r], cast to
    fp32 on device; the per-shift constant v is folded into g_e on the host.
    """
    import concourse.bacc as bacc
    import concourse.mybir as mybir
    from concourse.tile import TileContext

    fp32 = mybir.dt.float32
    i32 = mybir.dt.int32
    mm_dt = mybir.dt.float32r if use_fp32r else fp32
    OP = mybir.AluOpType

    nc = bacc.Bacc(None, target_bir_lowering=False)

    GE = 2 * BPC * NP
    XW = BPC * NH          # x0 stored once; both directions read the same view
    CW = GE + XW
    consts_d = nc.dram_tensor("consts", [NT, CW], fp32, kind="ExternalInput")
    out_d = nc.dram_tensor("out_all", [2, BPC, N], fp32, kind="ExternalOutput")

    (c1f, c1r), (wf, wr) = c1, w
    same_c1 = abs(c1f - c1r) < 1e-12
    same_w = all(abs(a - b) < 1e-12 for a, b in zip(wf, wr))

    g_off = 0
    x0_off = g_off + GE

    with TileContext(nc) as tc:
        with (
            tc.tile_pool(name="const", bufs=1) as cpool,
            tc.tile_pool(name="state", bufs=3) as spool,
            tc.tile_pool(name="work", bufs=3) as wpool,
            tc.tile_pool(name="accp", bufs=2) as apool,
            tc.tile_pool(name="psum", bufs=4, space="PSUM") as ppool,
        ):
            # constant loads issued from five different engine queues so the
            # issues don't serialize and the transfers run in parallel
            consts = cpool.tile([NT, CW], fp32, tag="consts")
            nc.scalar.dma_start(consts[:, x0_off:x0_off + XW],
                                consts_d[:, x0_off:x0_off + XW])
            nc.sync.dma_start(consts[:, g_off:g_off + GE],
                              consts_d[:, g_off:g_off + GE])

            # build the four 0/1 shift matrices on-device (no DMA latency):
            # vf[k,i] = (i-k) mod 128, vr[k,i] = (k-i) mod 128
            # Pf = [vf==1], Mf = [vf<2], Pr = [vr==1], Mr = [vr<2]
            matsf = cpool.tile([NT, 4 * NT], fp32, tag="matsf")
            vf = cpool.tile([NT, NT], i32, tag="vf")
            vr = cpool.tile([NT, NT], i32, tag="vr")
            nc.gpsimd.iota(vf[:], pattern=[[1, NT]], base=NT,
                           channel_multiplier=-1)
            nc.gpsimd.iota(vr[:], pattern=[[-1, NT]], base=NT,
                           channel_multiplier=1)
            nc.vector.tensor_scalar(vf[:], vf[:], scalar1=NT - 1, scalar2=None,
                                    op0=OP.bitwise_and)
            nc.vector.tensor_scalar(vr[:], vr[:], scalar1=NT - 1, scalar2=None,
                                    op0=OP.bitwise_and)
            nc.vector.tensor_scalar(matsf[:, 0:NT], vf[:], scalar1=1,
                                    scalar2=None, op0=OP.is_equal)
            nc.vector.tensor_scalar(matsf[:, 2 * NT:3 * NT], vr[:], scalar1=1,
                                    scalar2=None, op0=OP.is_equal)
            nc.vector.tensor_scalar(matsf[:, NT:2 * NT], vf[:], scalar1=2,
                                    scalar2=None, op0=OP.is_lt)
            nc.vector.tensor_scalar(matsf[:, 3 * NT:4 * NT], vr[:], scalar1=2,
                                    scalar2=None, op0=OP.is_lt)
            pm = {k: matsf[:, i * NT:(i + 1) * NT].bitcast(mm_dt)
                  for i, k in enumerate(("Pf", "Mf", "Pr", "Mr"))}
            g_e = consts[:, g_off:g_off + GE].rearrange(
                "t (d b p) -> t d b p", d=2, b=BPC)

            # initial state: host-packed with halos, read in place
            x0 = consts[:, x0_off:x0_off + XW].rearrange(
                "t (b p) -> t b p", b=BPC)

            # fully independent fwd / rev chains so one direction's DVE work
            # overlaps the other direction's matmuls
            DIRS = (
                dict(d=0, P="Pf", M="Mf", c1=c1f, w=wf, xtag="xf"),
                dict(d=1, P="Pr", M="Mr", c1=c1r, w=wr, xtag="xr"),
            )
            xs = [x0, x0]                     # [128, BPC, NH] views
            accs = [None, None]
            for s in range(S):
                for dd in DIRS:
                    d = dd["d"]
                    xd = xs[d]
                    # prop = P @ x + M @ xphi  (xphi: fwd cols 0..63, rev 2..65)
                    ps = ppool.tile([NT, BPC, NP], fp32, tag=f"ps{d}")
                    xm = xd.bitcast(mm_dt)
                    lo = 0 if d == 0 else 2
                    nc.tensor.matmul(ps[:], pm[dd["P"]], xm[:, :, 1:NP + 1],
                                     start=True, stop=False)
                    nc.tensor.matmul(ps[:], pm[dd["M"]], xm[:, :, lo:lo + NP],
                                     start=False, stop=True)

                    # tmp = prop * g_e ; x'_center = c1*x + tmp
                    tmp = wpool.tile([NT, BPC, NP], fp32, tag=f"tmp{d}")
                    nc.vector.tensor_mul(tmp[:], ps[:], g_e[:, d])
                    xn = spool.tile([NT, BPC, NH], fp32, tag=dd["xtag"])
                    xc = xn[:, :, 1:NP + 1]
                    nc.vector.scalar_tensor_tensor(
                        xc, xd[:, :, 1:NP + 1], dd["c1"], tmp[:],
                        op0=OP.mult, op1=OP.add)
                    # refresh wrap halos (tiny; last step never reads them)
                    if s < S - 1:
                        nc.gpsimd.tensor_copy(xn[:, :, 0:1], xn[:, :, NP:NP + 1])
                        nc.gpsimd.tensor_copy(xn[:, :, NP + 1:NP + 2], xn[:, :, 1:2])
                    xs[d] = xn[:]

                    # acc += w_s * x'
                    an = apool.tile([NT, BPC, NP], fp32, tag=f"acc{d}")
                    if accs[d] is None:
                        nc.vector.tensor_scalar_mul(an[:], xc, dd["w"][0])
                    else:
                        nc.vector.scalar_tensor_tensor(
                            an[:], xc, dd["w"][s], accs[d][:],
                            op0=OP.mult, op1=OP.add)
                    accs[d] = an

            # combine: inter = f*r ; comb = f + r + sig*inter
            # two output DMAs on different queues, each fired as soon as its
            # tensor is ready
            out2 = wpool.tile([NT, 2, BPC, NP], fp32, tag="out2")
            inter = out2[:, 1]
            nc.vector.tensor_mul(inter, accs[0][:], accs[1][:])
            fr = wpool.tile([NT, BPC, NP], fp32, tag="fr")
            nc.vector.tensor_add(fr[:], accs[0][:], accs[1][:])
            ov = out_d[:].rearrange("o b (t p) -> o t b p", t=NT)
            nc.scalar.dma_start(ov[1], inter)
            nc.vector.scalar_tensor_tensor(
                out2[:, 0], inter, sig_w, fr[:], op0=OP.mult, op1=OP.add)
            nc.sync.dma_start(ov[0], out2[:, 0])

    nc.finalize()
    return nc


def _host_prep(inputs):
    entry = np.ascontiguousarray(np.asarray(inputs["entry_probs"], np.float32))
    fwd_adj = np.asarray(inputs["forward_adj"], np.float32)
    rev_adj = np.asarray(inputs["reverse_adj"], np.float32)
    angles = np.asarray(inputs["bounce_angles"], np.float32)

    vf = _diag_vals(fwd_adj, _FWD)
    vr = _diag_vals(rev_adj, _REV)
    ok = _structure_ok(fwd_adj, vf) and _structure_ok(rev_adj, vr)

    df = float(np.clip(float(np.asarray(inputs["forward_decay"])), 0.5, 0.99))
    dr = float(np.clip(float(np.asarray(inputs["reverse_decay"])), 0.5, 0.99))
    wf = _softmax(np.asarray(inputs["forward_step_weights"], np.float32))
    wr = _softmax(np.asarray(inputs["reverse_step_weights"], np.float32))
    sig = float(1.0 / (1.0 + np.exp(-float(np.asarray(inputs["interaction_weight"])))))

    vbf = [float(v.mean()) for v in vf]   # [v10, v01, v11]
    vbr = [float(v.mean()) for v in vr]
    # 0/1 uint8 matrices require one shared constant per direction
    for vs in (vbf, vbr):
        if abs(vs[0] - vs[1]) > 1e-6 * abs(vs[0]) or \
           abs(vs[0] - vs[2]) > 1e-6 * abs(vs[0]):
            ok = False

    # per-cell gain g = v * 0.7 * decay * (0.5 + 0.5*cos(mean|angles|)),
    # expanded to [128, dir, b, 64]; v folded here since matrices are 0/1
    af = (0.5 + 0.5 * np.cos(np.abs(angles).mean(axis=1))).astype(np.float32)
    af2 = af.reshape(NT, NP)
    g_e = np.empty((NT, 2, BPC, NP), np.float32)
    g_e[:, 0] = (0.7 * df * vbf[0]) * af2[:, None, :]
    g_e[:, 1] = (0.7 * dr * vbr[0]) * af2[:, None, :]

    consts_common = g_e.reshape(NT, -1)

    consts_list = []
    for c in range(NCORES):
        e = entry[c * BPC:(c + 1) * BPC].reshape(BPC, NT, NP).transpose(1, 0, 2)
        x0 = np.empty((NT, BPC, NH), np.float32)
        x0[:, :, 1:NP + 1] = e
        x0[:, :, 0] = e[:, :, NP - 1]
        x0[:, :, NP + 1] = e[:, :, 0]
        consts_list.append(np.ascontiguousarray(
            np.concatenate([consts_common, x0.reshape(NT, -1)], axis=1)))

    meta = dict(
        ok=ok,
        c1=(0.3 * df, 0.3 * dr),
        w=(list(map(float, wf)), list(map(float, wr))),
        sig=sig, consts_list=consts_list,
    )
    return meta


_PROGRAM_CACHE = {}
LAST_RESULT = None
USE_FP32R = False


def kernel(**inputs):
    meta = _host_prep(inputs)
    if not meta["ok"]:
        return _reference_fallback(
            np.asarray(inputs["entry_probs"], np.float32),
            np.asarray(inputs["forward_adj"], np.float32),
            np.asarray(inputs["reverse_adj"], np.float32),
            inputs["forward_step_weights"], inputs["forward_decay"],
            inputs["reverse_step_weights"], inputs["reverse_decay"],
            inputs["interaction_weight"], np.asarray(inputs["bounce_angles"], np.float32))

    from concourse import bass_utils

    key = (tuple(meta["c1"]), tuple(meta["w"][0]), tuple(meta["w"][1]),
           meta["sig"], USE_FP32R)
    if key not in _PROGRAM_CACHE:
        _PROGRAM_CACHE[key] = _build_program(
            meta["c1"], meta["w"], meta["sig"], use_fp32r=USE_FP32R)
    nc = _PROGRAM_CACHE[key]

    in_maps = [{"consts": meta["consts_list"][c]} for c in range(NCORES)]
    res = bass_utils.run_bass_kernel_spmd(nc, in_maps, core_ids=list(range(NCORES)))
    global LAST_RESULT
    LAST_RESULT = res

    combined = np.concatenate([r["out_all"][0] for r in res.results], axis=0)
    interaction = np.concatenate([r["out_all"][1] for r in res.results], axis=0)
    return combined, interaction
